# revision 16
# baseline (speedup 1.0000x reference)
"""Trainium2 Bass kernel for nn_Head (sparse attention head).

Computation (per batch b):
    K = X @ Wk; Q = X @ Wq; V = X @ Wv                       # [T, HS]
    S = Q K^T / sqrt(HS)                                     # [T, T]
    A = softmax_row(where(dag==0, -inf, S))                  # row-wise over keys
    out[j, h] = sum_i A[i, j] V[i, h]   (transposed AV)      # [T, HS]
    return swish(out)

Sharding over 8 NeuronCores: core = (b, h) with b = batch (4), h = query-row
half (2).  Each core computes its 2048-query slice.

The dag mask is applied ADDITIVELY before exp: host sends mb = -192 where
dag==0 else 0 (exact in fp8e4), and an identity-stationary matmul accumulates
mb into the score PSUM, so exp(S/8 + mb/8) ~ exp(S/8) * [dag!=0] with masked
terms at exp(-18.5+*) ~ 1e-8 (negligible vs row sums ~3e3).  The row-sum
needed for the softmax denominator comes free from the activation accum_out.
The denominator is folded into the V stationary operand vt = V/l * 1024, and
OT_partial[h, j] = sum_{i in shard} u[i,j] * vt[i,h].  Host sums the two
partials per batch, divides by 1024, transposes, applies swish.
"""

import sys

for _p in ("/opt/trn_rl_repo",):
    if _p not in sys.path:
        sys.path.append(_p)

import numpy as np
import ml_dtypes

import concourse.bacc as bacc
import concourse.mybir as mybir
import concourse.tile as tile
from concourse.bass_utils import run_bass_kernel_spmd

B, T, D, HS = 4, 4096, 512, 64
TH = T // 2          # query rows per core
P = 128              # partitions
NB = TH // P         # 16 i-blocks per core
NCC = D // P         # 4 contraction chunks over D
NJ = 512             # matmul moving free dim (1 PSUM bank of f32)
VSCALE = 1024.0      # fp16 dynamic-range scale folded into V/l
MBIAS = -192.0       # additive mask bias (exact in fp8e4); *0.125 => -24

F8 = mybir.dt.float8e4
F16 = mybir.dt.float16
F32 = mybir.dt.float32
AF = mybir.ActivationFunctionType
ALU = mybir.AluOpType

_CACHE = {}


def _build():
    if "nc" in _CACHE:
        return _CACHE["nc"]

    nc = bacc.Bacc("TRN2", target_bir_lowering=False, debug=False)

    xt_d = nc.dram_tensor("xt", [D, T], F16, kind="ExternalInput").ap()
    mb_d = nc.dram_tensor("mb", [TH, T], F8, kind="ExternalInput").ap()
    id_d = nc.dram_tensor("idt", [P, P], F8, kind="ExternalInput").ap()
    wk_d = nc.dram_tensor("wk", [D, HS], F16, kind="ExternalInput").ap()
    wq_d = nc.dram_tensor("wq", [D, HS], F16, kind="ExternalInput").ap()
    wv_d = nc.dram_tensor("wv", [D, HS], F16, kind="ExternalInput").ap()
    ot_d = nc.dram_tensor("ot", [HS, T], F32, kind="ExternalOutput").ap()

    with tile.TileContext(nc) as tc:
        with tc.tile_pool(name="persist", bufs=1) as pp:
            kt = pp.tile([HS, T], F16, tag="kt")         # K^T
            qt = pp.tile([HS, TH], F16, tag="qt")        # Q^T (shard rows)
            v = pp.tile([P, NB * HS], F16, tag="v")      # V rows (shard)
            vt = pp.tile([P, NB * HS], F16, tag="vt")    # V/l * VSCALE
            idt = pp.tile([P, P], F8, tag="idt")         # fp8 identity
            u = pp.tile([P, NB * T], F16, tag="u")       # masked exp(S/8)

            nc.sync.dma_start(idt[:], id_d[:, :])

            # ---- phase A: load X^T / weights, compute K^T, Q^T, V ----
            with (
                tc.tile_pool(name="phA", bufs=1) as pA,
                tc.tile_pool(name="psA", bufs=2, space="PSUM") as psA,
                tc.tile_pool(name="warm", bufs=1) as pW,
            ):
                xt = pA.tile([P, NCC * T], F16, tag="xt")
                wk = pA.tile([P, NCC * HS], F16, tag="wk")
                wq = pA.tile([P, NCC * HS], F16, tag="wq")
                wv = pA.tile([P, NCC * HS], F16, tag="wv")
                # wk first on gpsimd (kt g0 needs it), then xt pieces
                # round-robined across the three DMA-capable engine queues
                # in consumption (jq-major) order
                for ci in range(NCC):
                    cs = slice(ci * P, (ci + 1) * P)
                    nc.gpsimd.dma_start(wk[:, ci * HS:(ci + 1) * HS], wk_d[cs, :])
                engs = (nc.sync, nc.scalar, nc.gpsimd)
                for jq in range(4):
                    for ci in range(NCC):
                        engs[(jq * NCC + ci) % 3].dma_start(
                            xt[:, ci * T + jq * 1024: ci * T + (jq + 1) * 1024],
                            xt_d[ci * P:(ci + 1) * P, jq * 1024:(jq + 1) * 1024],
                        )
                for ci in range(NCC):
                    cs = slice(ci * P, (ci + 1) * P)
                    nc.scalar.dma_start(wq[:, ci * HS:(ci + 1) * HS], wq_d[cs, :])
                    nc.scalar.dma_start(wv[:, ci * HS:(ci + 1) * HS], wv_d[cs, :])

                # warm the Exp table + the PE HAM clock-gate while DMAs run
                wsrc = pW.tile([P, NJ], F16, tag="wsrc")
                wdst = pW.tile([P, 8], F16, tag="wdst")
                wps = psA.tile([P, NJ], F32, tag="wps")
                nc.vector.memset(wsrc[:], 0.0)
                nc.scalar.activation(wdst[:], wsrc[:, 0:8], AF.Exp, scale=1.0)

                def warm_mm(n):
                    for _ in range(n):
                        nc.tensor.matmul(
                            wps[:], wsrc[:, 0:P], wsrc[:], start=True, stop=True
                        )

                warm_mm(12)
                for j0 in range(0, T, NJ):
                    ktp = psA.tile([HS, NJ], F32, tag="ktp")
                    for ci in range(NCC):
                        nc.tensor.matmul(
                            ktp[:],
                            wk[:, ci * HS:(ci + 1) * HS],
                            xt[:, ci * T + j0: ci * T + j0 + NJ],
                            start=(ci == 0),
                            stop=(ci == NCC - 1),
                        )
                    nc.vector.tensor_copy(kt[:, j0:j0 + NJ], ktp[:])
                    if j0 < T - 2 * NJ:
                        warm_mm(2)

                for j0 in range(0, TH, NJ):
                    qtp = psA.tile([HS, NJ], F32, tag="ktp")
                    for ci in range(NCC):
                        nc.tensor.matmul(
                            qtp[:],
                            wq[:, ci * HS:(ci + 1) * HS],
                            xt[:, ci * T + j0: ci * T + j0 + NJ],
                            start=(ci == 0),
                            stop=(ci == NCC - 1),
                        )
                    nc.vector.tensor_copy(qt[:, j0:j0 + NJ], qtp[:])

                for k in range(NB):
                    vp = psA.tile([P, HS], F32, tag="vp")
                    for ci in range(NCC):
                        nc.tensor.matmul(
                            vp[:],
                            xt[:, ci * T + k * P: ci * T + (k + 1) * P],
                            wv[:, ci * HS:(ci + 1) * HS],
                            start=(ci == 0),
                            stop=(ci == NCC - 1),
                        )
                    nc.vector.tensor_copy(v[:, k * HS:(k + 1) * HS], vp[:])

            # ---- phase B: scores + mask-bias inject, exp w/ accum ----
            with (
                tc.tile_pool(name="pmb", bufs=3) as pmb,
                tc.tile_pool(name="pBl", bufs=2) as pBl,
                tc.tile_pool(name="psB", bufs=2, space="PSUM") as psB,
            ):
                mb_tiles = []

                def mb_fetch(kk):
                    t = pmb.tile([P, T], F8, tag="mb")
                    nc.gpsimd.dma_start(t[:, 0:TH], mb_d[kk * P:(kk + 1) * P, 0:TH])
                    nc.sync.dma_start(t[:, TH:T], mb_d[kk * P:(kk + 1) * P, TH:T])
                    mb_tiles.append(t)

                mb_fetch(0)
                mb_fetch(1)
                for k in range(NB):
                    if k + 2 < NB:
                        mb_fetch(k + 2)
                    mb = mb_tiles[k]
                    l_halves = []
                    for jh in range(2):
                        sp = psB.tile([P, TH], F32, tag="s")
                        for c in range(4):
                            nc.tensor.matmul(
                                sp[:, c * NJ:(c + 1) * NJ],
                                qt[:, k * P:(k + 1) * P],
                                kt[:, jh * TH + c * NJ: jh * TH + (c + 1) * NJ],
                                start=True,
                                stop=False,
                            )
                        for c in range(4):
                            nc.tensor.matmul(
                                sp[:, c * NJ:(c + 1) * NJ],
                                idt[:],
                                mb[:, jh * TH + c * NJ: jh * TH + (c + 1) * NJ],
                                start=False,
                                stop=True,
                            )
                        l_acc = pBl.tile([P, 1], F32, tag=f"l{jh}")
                        nc.scalar.activation(
                            u[:, k * T + jh * TH: k * T + (jh + 1) * TH],
                            sp[:],
                            AF.Exp,
                            scale=0.125,
                            accum_out=l_acc[:],
                        )
                        l_halves.append(l_acc)
                    l_tot = pBl.tile([P, 1], F32, tag="lt")
                    nc.vector.tensor_tensor(
                        out=l_tot[:], in0=l_halves[0][:], in1=l_halves[1][:],
                        op=ALU.add,
                    )
                    rl = pBl.tile([P, 1], F32, tag="rl")
                    nc.vector.reciprocal(rl[:], l_tot[:])
                    nc.vector.tensor_scalar(
                        out=vt[:, k * HS:(k + 1) * HS],
                        in0=v[:, k * HS:(k + 1) * HS],
                        scalar1=rl[:],
                        scalar2=VSCALE,
                        op0=ALU.mult,
                        op1=ALU.mult,
                    )

            # ---- phase C: OT = sum_k vt_k^T . u_k  (transposed AV) ----
            # j-chunk outer so each chunk's copy + store pipelines behind
            # the remaining chunks' matmuls.
            with (
                tc.tile_pool(name="psC", bufs=1, space="PSUM") as psC,
                tc.tile_pool(name="phC", bufs=1) as pC,
            ):
                ot = psC.tile([HS, T], F32, tag="ot")
                ot_sb = pC.tile([HS, T], F32, tag="ot_sb")
                for jq in range(T // NJ):
                    js = slice(jq * NJ, (jq + 1) * NJ)
                    for k in range(NB):
                        nc.tensor.matmul(
                            ot[:, js],
                            vt[:, k * HS:(k + 1) * HS],
                            u[:, k * T + jq * NJ: k * T + (jq + 1) * NJ],
                            start=(k == 0),
                            stop=(k == NB - 1),
                        )
                    if jq % 2 == 0:
                        nc.scalar.copy(ot_sb[:, js], ot[:, js])
                    else:
                        nc.vector.tensor_copy(ot_sb[:, js], ot[:, js])
                    if jq % 2 == 0:
                        nc.sync.dma_start(ot_d[:, js], ot_sb[:, js])
                    else:
                        nc.scalar.dma_start(ot_d[:, js], ot_sb[:, js])

    nc.compile()
    _CACHE["nc"] = nc
    return nc


def _prep_inputs(X, dag, Wk, Wq, Wv):
    X = np.asarray(X, dtype=np.float32)
    dag = np.asarray(dag)
    w16 = {
        "wk": np.asarray(Wk, dtype=np.float16),
        "wq": np.asarray(Wq, dtype=np.float16),
        "wv": np.asarray(Wv, dtype=np.float16),
    }
    idt = np.eye(P, dtype=ml_dtypes.float8_e4m3)
    mbias = np.where(dag == 0, np.float32(MBIAS), np.float32(0.0)).astype(
        ml_dtypes.float8_e4m3
    )
    # column-permuted views: the core's own query-half tokens come first,
    # so qt/v matmuls can read fixed xt columns in the shared SPMD program.
    mb_perm = [
        np.ascontiguousarray(
            np.concatenate(
                [
                    mbias[h * TH:(h + 1) * TH, h * TH:(h + 1) * TH],
                    mbias[h * TH:(h + 1) * TH, (1 - h) * TH:(2 - h) * TH],
                ],
                axis=1,
            )
        )
        for h in range(2)
    ]
    in_maps = []
    for core in range(8):
        b, h = divmod(core, 2)
        xb = X[b].astype(np.float16)
        xb2 = np.concatenate(
            [xb[h * TH:(h + 1) * TH], xb[(1 - h) * TH:(2 - h) * TH]], axis=0
        )
        in_maps.append(
            {
                "xt": np.ascontiguousarray(xb2.T),
                "mb": mb_perm[h],
                "idt": idt,
                **w16,
            }
        )
    return in_maps


def kernel(X, dag, Wk, Wq, Wv, _trace=False):
    nc = _build()
    in_maps = _prep_inputs(X, dag, Wk, Wq, Wv)
    res = run_bass_kernel_spmd(nc, in_maps, list(range(8)), trace=_trace)
    out = np.empty((B, T, HS), dtype=np.float32)
    for b in range(B):
        ot0 = res.results[2 * b]["ot"]          # h=0: columns already global
        ot1 = res.results[2 * b + 1]["ot"]      # h=1: halves swapped
        ot = ot0.copy()
        ot[:, 0:TH] += ot1[:, TH:T]
        ot[:, TH:T] += ot1[:, 0:TH]
        o = ot.T / np.float32(VSCALE)
        out[b] = o / (1.0 + np.exp(-o))  # swish: o * sigmoid(o)
    if _trace:
        return out, res
    return out


# revision 17
# speedup vs baseline: 1.0187x; 1.0187x over previous
"""Trainium2 Bass kernel for nn_Head (sparse attention head).

Computation (per batch b):
    K = X @ Wk; Q = X @ Wq; V = X @ Wv                       # [T, HS]
    S = Q K^T / sqrt(HS)                                     # [T, T]
    A = softmax_row(where(dag==0, -inf, S))                  # row-wise over keys
    out[j, h] = sum_i A[i, j] V[i, h]   (transposed AV)      # [T, HS]
    return swish(out)

Sharding over 8 NeuronCores: core = (b, h) with b = batch (4), h = query-row
half (2).  Each core computes its 2048-query slice.

The dag mask is applied ADDITIVELY before exp: host sends mb = -192 where
dag==0 else 0 (exact in fp8e4), and an identity-stationary matmul accumulates
mb into the score PSUM, so exp(S/8 + mb/8) ~ exp(S/8) * [dag!=0] with masked
terms at exp(-18.5+*) ~ 1e-8 (negligible vs row sums ~3e3).  The row-sum
needed for the softmax denominator comes free from the activation accum_out.
The denominator is folded into the V stationary operand vt = V/l * 1024, and
OT_partial[h, j] = sum_{i in shard} u[i,j] * vt[i,h].  Host sums the two
partials per batch, divides by 1024, transposes, applies swish.
"""

import sys

for _p in ("/opt/trn_rl_repo",):
    if _p not in sys.path:
        sys.path.append(_p)

import numpy as np
import ml_dtypes

import concourse.bacc as bacc
import concourse.mybir as mybir
import concourse.tile as tile
from concourse.bass_utils import run_bass_kernel_spmd

B, T, D, HS = 4, 4096, 512, 64
TH = T // 2          # query rows per core
P = 128              # partitions
NB = TH // P         # 16 i-blocks per core
NCC = D // P         # 4 contraction chunks over D
NJ = 512             # matmul moving free dim (1 PSUM bank of f32)
VSCALE = 1024.0      # fp16 dynamic-range scale folded into V/l
MBIAS = -192.0       # additive mask bias (exact in fp8e4); *0.125 => -24

F8 = mybir.dt.float8e4
F16 = mybir.dt.float16
F32 = mybir.dt.float32
AF = mybir.ActivationFunctionType
ALU = mybir.AluOpType

_CACHE = {}


def _build():
    if "nc" in _CACHE:
        return _CACHE["nc"]

    nc = bacc.Bacc("TRN2", target_bir_lowering=False, debug=False)

    xt_d = nc.dram_tensor("xt", [D, T], F16, kind="ExternalInput").ap()
    mb_d = nc.dram_tensor("mb", [TH, T], F8, kind="ExternalInput").ap()
    id_d = nc.dram_tensor("idt", [P, P], F8, kind="ExternalInput").ap()
    wk_d = nc.dram_tensor("wk", [D, HS], F16, kind="ExternalInput").ap()
    wq_d = nc.dram_tensor("wq", [D, HS], F16, kind="ExternalInput").ap()
    wv_d = nc.dram_tensor("wv", [D, HS], F16, kind="ExternalInput").ap()
    ot_d = nc.dram_tensor("ot", [HS, T], F32, kind="ExternalOutput").ap()

    with tile.TileContext(nc) as tc:
        with tc.tile_pool(name="persist", bufs=1) as pp:
            kt = pp.tile([HS, T], F16, tag="kt")         # K^T
            qt = pp.tile([HS, TH], F16, tag="qt")        # Q^T (shard rows)
            v = pp.tile([P, NB * HS], F16, tag="v")      # V rows (shard)
            vt = pp.tile([P, NB * HS], F16, tag="vt")    # V/l * VSCALE
            idt = pp.tile([P, P], F8, tag="idt")         # fp8 identity
            u = pp.tile([P, NB * T], F16, tag="u")       # masked exp(S/8)

            nc.sync.dma_start(idt[:], id_d[:, :])

            # ---- phase A: load X^T / weights, compute K^T, Q^T, V ----
            with (
                tc.tile_pool(name="phA", bufs=1) as pA,
                tc.tile_pool(name="psA", bufs=2, space="PSUM") as psA,
                tc.tile_pool(name="warm", bufs=1) as pW,
            ):
                xt = pA.tile([P, NCC * T], F16, tag="xt")
                wk = pA.tile([P, NCC * HS], F16, tag="wk")
                wq = pA.tile([P, NCC * HS], F16, tag="wq")
                wv = pA.tile([P, NCC * HS], F16, tag="wv")
                # wk first on gpsimd (kt g0 needs it), then xt pieces
                # round-robined across the three DMA-capable engine queues
                # in consumption (jq-major) order
                for ci in range(NCC):
                    cs = slice(ci * P, (ci + 1) * P)
                    nc.gpsimd.dma_start(wk[:, ci * HS:(ci + 1) * HS], wk_d[cs, :])
                for jq in range(4):
                    for ci in range(NCC):
                        eng = nc.sync if ci % 2 == 0 else nc.scalar
                        eng.dma_start(
                            xt[:, ci * T + jq * 1024: ci * T + (jq + 1) * 1024],
                            xt_d[ci * P:(ci + 1) * P, jq * 1024:(jq + 1) * 1024],
                        )
                for ci in range(NCC):
                    cs = slice(ci * P, (ci + 1) * P)
                    nc.scalar.dma_start(wq[:, ci * HS:(ci + 1) * HS], wq_d[cs, :])
                    nc.scalar.dma_start(wv[:, ci * HS:(ci + 1) * HS], wv_d[cs, :])

                # warm the Exp table + the PE HAM clock-gate while DMAs run
                wsrc = pW.tile([P, NJ], F16, tag="wsrc")
                wdst = pW.tile([P, 8], F16, tag="wdst")
                wps = psA.tile([P, NJ], F32, tag="wps")
                nc.vector.memset(wsrc[:], 0.0)
                nc.scalar.activation(wdst[:], wsrc[:, 0:8], AF.Exp, scale=1.0)

                def warm_mm(n):
                    for _ in range(n):
                        nc.tensor.matmul(
                            wps[:], wsrc[:, 0:P], wsrc[:], start=True, stop=True
                        )

                warm_mm(12)
                for j0 in range(0, T, NJ):
                    ktp = psA.tile([HS, NJ], F32, tag="ktp")
                    for ci in range(NCC):
                        nc.tensor.matmul(
                            ktp[:],
                            wk[:, ci * HS:(ci + 1) * HS],
                            xt[:, ci * T + j0: ci * T + j0 + NJ],
                            start=(ci == 0),
                            stop=(ci == NCC - 1),
                        )
                    nc.vector.tensor_copy(kt[:, j0:j0 + NJ], ktp[:])
                    if j0 < T - 2 * NJ:
                        warm_mm(2)

                for j0 in range(0, TH, NJ):
                    qtp = psA.tile([HS, NJ], F32, tag="ktp")
                    for ci in range(NCC):
                        nc.tensor.matmul(
                            qtp[:],
                            wq[:, ci * HS:(ci + 1) * HS],
                            xt[:, ci * T + j0: ci * T + j0 + NJ],
                            start=(ci == 0),
                            stop=(ci == NCC - 1),
                        )
                    nc.vector.tensor_copy(qt[:, j0:j0 + NJ], qtp[:])

                for k in range(NB):
                    vp = psA.tile([P, HS], F32, tag="vp")
                    for ci in range(NCC):
                        nc.tensor.matmul(
                            vp[:],
                            xt[:, ci * T + k * P: ci * T + (k + 1) * P],
                            wv[:, ci * HS:(ci + 1) * HS],
                            start=(ci == 0),
                            stop=(ci == NCC - 1),
                        )
                    nc.vector.tensor_copy(v[:, k * HS:(k + 1) * HS], vp[:])

            # ---- phase B: scores + mask-bias inject, exp w/ accum ----
            with (
                tc.tile_pool(name="pmb", bufs=6) as pmb,
                tc.tile_pool(name="pBl", bufs=2) as pBl,
                tc.tile_pool(name="psB", bufs=2, space="PSUM") as psB,
            ):
                for k in range(NB):
                    mb = pmb.tile([P, T], F8, tag="mb")
                    nc.gpsimd.dma_start(mb[:, 0:TH], mb_d[k * P:(k + 1) * P, 0:TH])
                    nc.sync.dma_start(mb[:, TH:T], mb_d[k * P:(k + 1) * P, TH:T])
                    l_halves = []
                    for jh in range(2):
                        sp = psB.tile([P, TH], F32, tag="s")
                        for c in range(4):
                            nc.tensor.matmul(
                                sp[:, c * NJ:(c + 1) * NJ],
                                qt[:, k * P:(k + 1) * P],
                                kt[:, jh * TH + c * NJ: jh * TH + (c + 1) * NJ],
                                start=True,
                                stop=False,
                            )
                        for c in range(4):
                            nc.tensor.matmul(
                                sp[:, c * NJ:(c + 1) * NJ],
                                idt[:],
                                mb[:, jh * TH + c * NJ: jh * TH + (c + 1) * NJ],
                                start=False,
                                stop=True,
                            )
                        l_acc = pBl.tile([P, 1], F32, tag=f"l{jh}")
                        nc.scalar.activation(
                            u[:, k * T + jh * TH: k * T + (jh + 1) * TH],
                            sp[:],
                            AF.Exp,
                            scale=0.125,
                            accum_out=l_acc[:],
                        )
                        l_halves.append(l_acc)
                    l_tot = pBl.tile([P, 1], F32, tag="lt")
                    nc.vector.tensor_tensor(
                        out=l_tot[:], in0=l_halves[0][:], in1=l_halves[1][:],
                        op=ALU.add,
                    )
                    rl = pBl.tile([P, 1], F32, tag="rl")
                    nc.vector.reciprocal(rl[:], l_tot[:])
                    nc.vector.tensor_scalar(
                        out=vt[:, k * HS:(k + 1) * HS],
                        in0=v[:, k * HS:(k + 1) * HS],
                        scalar1=rl[:],
                        scalar2=VSCALE,
                        op0=ALU.mult,
                        op1=ALU.mult,
                    )

            # ---- phase C: OT = sum_k vt_k^T . u_k  (transposed AV) ----
            # j-chunk outer so each chunk's copy + store pipelines behind
            # the remaining chunks' matmuls.
            with (
                tc.tile_pool(name="psC", bufs=1, space="PSUM") as psC,
                tc.tile_pool(name="phC", bufs=1) as pC,
            ):
                ot = psC.tile([HS, T], F32, tag="ot")
                ot_sb = pC.tile([HS, T], F32, tag="ot_sb")
                for jq in range(T // NJ):
                    js = slice(jq * NJ, (jq + 1) * NJ)
                    for k in range(NB):
                        nc.tensor.matmul(
                            ot[:, js],
                            vt[:, k * HS:(k + 1) * HS],
                            u[:, k * T + jq * NJ: k * T + (jq + 1) * NJ],
                            start=(k == 0),
                            stop=(k == NB - 1),
                        )
                    if jq % 2 == 0:
                        nc.scalar.copy(ot_sb[:, js], ot[:, js])
                    else:
                        nc.vector.tensor_copy(ot_sb[:, js], ot[:, js])
                    if jq % 2 == 0:
                        nc.sync.dma_start(ot_d[:, js], ot_sb[:, js])
                    else:
                        nc.scalar.dma_start(ot_d[:, js], ot_sb[:, js])

    nc.compile()
    _CACHE["nc"] = nc
    return nc


def _prep_inputs(X, dag, Wk, Wq, Wv):
    X = np.asarray(X, dtype=np.float32)
    dag = np.asarray(dag)
    w16 = {
        "wk": np.asarray(Wk, dtype=np.float16),
        "wq": np.asarray(Wq, dtype=np.float16),
        "wv": np.asarray(Wv, dtype=np.float16),
    }
    idt = np.eye(P, dtype=ml_dtypes.float8_e4m3)
    mbias = np.where(dag == 0, np.float32(MBIAS), np.float32(0.0)).astype(
        ml_dtypes.float8_e4m3
    )
    # column-permuted views: the core's own query-half tokens come first,
    # so qt/v matmuls can read fixed xt columns in the shared SPMD program.
    mb_perm = [
        np.ascontiguousarray(
            np.concatenate(
                [
                    mbias[h * TH:(h + 1) * TH, h * TH:(h + 1) * TH],
                    mbias[h * TH:(h + 1) * TH, (1 - h) * TH:(2 - h) * TH],
                ],
                axis=1,
            )
        )
        for h in range(2)
    ]
    in_maps = []
    for core in range(8):
        b, h = divmod(core, 2)
        xb = X[b].astype(np.float16)
        xb2 = np.concatenate(
            [xb[h * TH:(h + 1) * TH], xb[(1 - h) * TH:(2 - h) * TH]], axis=0
        )
        in_maps.append(
            {
                "xt": np.ascontiguousarray(xb2.T),
                "mb": mb_perm[h],
                "idt": idt,
                **w16,
            }
        )
    return in_maps


def kernel(X, dag, Wk, Wq, Wv, _trace=False):
    nc = _build()
    in_maps = _prep_inputs(X, dag, Wk, Wq, Wv)
    res = run_bass_kernel_spmd(nc, in_maps, list(range(8)), trace=_trace)
    out = np.empty((B, T, HS), dtype=np.float32)
    for b in range(B):
        ot0 = res.results[2 * b]["ot"]          # h=0: columns already global
        ot1 = res.results[2 * b + 1]["ot"]      # h=1: halves swapped
        ot = ot0.copy()
        ot[:, 0:TH] += ot1[:, TH:T]
        ot[:, TH:T] += ot1[:, 0:TH]
        o = ot.T / np.float32(VSCALE)
        out[b] = o / (1.0 + np.exp(-o))  # swish: o * sigmoid(o)
    if _trace:
        return out, res
    return out


# revision 18
# speedup vs baseline: 1.2816x; 1.2580x over previous
"""Trainium2 Bass kernel for nn_Head (sparse attention head).

Computation (per batch b):
    K = X @ Wk; Q = X @ Wq; V = X @ Wv                       # [T, HS]
    S = Q K^T / sqrt(HS)                                     # [T, T]
    A = softmax_row(where(dag==0, -inf, S))                  # row-wise over keys
    out[j, h] = sum_i A[i, j] V[i, h]   (transposed AV)      # [T, HS]
    return swish(out)

Sharding over 8 NeuronCores: core = (b, h) with b = batch (4), h = query-row
half (2).  Each core computes its 2048-query slice.

The dag mask is applied ADDITIVELY before exp: host sends mb = -192 where
dag==0 else 0 (exact in fp8e4), and an identity-stationary matmul accumulates
mb into the score PSUM, so exp(S/8 + mb/8) ~ exp(S/8) * [dag!=0] with masked
terms at exp(-18.5+*) ~ 1e-8 (negligible vs row sums ~3e3).  The row-sum
needed for the softmax denominator comes free from the activation accum_out.
The denominator is folded into the V stationary operand vt = V/l * 1024, and
OT_partial[h, j] = sum_{i in shard} u[i,j] * vt[i,h].  Host sums the two
partials per batch, divides by 1024, transposes, applies swish.
"""

import sys

for _p in ("/opt/trn_rl_repo",):
    if _p not in sys.path:
        sys.path.append(_p)

import numpy as np
import ml_dtypes

import concourse.bacc as bacc
import concourse.mybir as mybir
import concourse.tile as tile
from concourse.bass_utils import run_bass_kernel_spmd

B, T, D, HS = 4, 4096, 512, 64
TH = T // 2          # query rows per core
P = 128              # partitions
NB = TH // P         # 16 i-blocks per core
NCC = D // P         # 4 contraction chunks over D
NJ = 512             # matmul moving free dim (1 PSUM bank of f32)
VSCALE = 1024.0      # fp16 dynamic-range scale folded into V/l
MBIAS = -192.0       # additive mask bias (exact in fp8e4); *0.125 => -24

F8 = mybir.dt.float8e4
F16 = mybir.dt.float16
F32 = mybir.dt.float32
AF = mybir.ActivationFunctionType
ALU = mybir.AluOpType

_CACHE = {}


def _build():
    if "nc" in _CACHE:
        return _CACHE["nc"]

    nc = bacc.Bacc("TRN2", target_bir_lowering=False, debug=False)

    xt_d = nc.dram_tensor("xt", [D, T], F16, kind="ExternalInput").ap()
    mb_d = nc.dram_tensor("mb", [TH, T], F8, kind="ExternalInput").ap()
    id_d = nc.dram_tensor("idt", [P, P], F8, kind="ExternalInput").ap()
    id16_d = nc.dram_tensor("id16", [HS, HS], F16, kind="ExternalInput").ap()
    wk_d = nc.dram_tensor("wk", [D, HS], F16, kind="ExternalInput").ap()
    wq_d = nc.dram_tensor("wq", [D, HS], F16, kind="ExternalInput").ap()
    wv_d = nc.dram_tensor("wv", [D, HS], F16, kind="ExternalInput").ap()
    ot_d = nc.dram_tensor("ot", [HS, T], F32, kind="ExternalOutput").ap()

    with tile.TileContext(nc) as tc:
        with tc.tile_pool(name="persist", bufs=1) as pp:
            kt = pp.tile([HS, T], F16, tag="kt")         # K^T
            qt = pp.tile([HS, TH], F16, tag="qt")        # Q^T (shard rows)
            v = pp.tile([P, NB * HS], F16, tag="v")      # V rows (shard)
            vt = pp.tile([P, NB * HS], F16, tag="vt")    # V/l * VSCALE
            idt = pp.tile([P, P], F8, tag="idt")         # fp8 identity
            id16 = pp.tile([HS, HS], F16, tag="id16")    # f16 identity (transpose)
            u = pp.tile([P, NB * T], F16, tag="u")       # masked exp(S/8)

            nc.sync.dma_start(idt[:], id_d[:, :])
            nc.sync.dma_start(id16[:], id16_d[:, :])

            # ---- phase A: load X^T / weights, compute K^T, Q^T, V ----
            with (
                tc.tile_pool(name="phA", bufs=1) as pA,
                tc.tile_pool(name="psA", bufs=2, space="PSUM") as psA,
                tc.tile_pool(name="warm", bufs=1) as pW,
            ):
                xt = pA.tile([P, NCC * T], F16, tag="xt")
                wk = pA.tile([P, NCC * HS], F16, tag="wk")
                wq = pA.tile([P, NCC * HS], F16, tag="wq")
                wv = pA.tile([P, NCC * HS], F16, tag="wv")
                # wk first on gpsimd (kt g0 needs it), then xt pieces
                # round-robined across the three DMA-capable engine queues
                # in consumption (jq-major) order
                for ci in range(NCC):
                    cs = slice(ci * P, (ci + 1) * P)
                    nc.gpsimd.dma_start(wk[:, ci * HS:(ci + 1) * HS], wk_d[cs, :])
                for jq in range(4):
                    for ci in range(NCC):
                        eng = nc.sync if ci % 2 == 0 else nc.scalar
                        eng.dma_start(
                            xt[:, ci * T + jq * 1024: ci * T + (jq + 1) * 1024],
                            xt_d[ci * P:(ci + 1) * P, jq * 1024:(jq + 1) * 1024],
                        )
                for ci in range(NCC):
                    cs = slice(ci * P, (ci + 1) * P)
                    nc.scalar.dma_start(wq[:, ci * HS:(ci + 1) * HS], wq_d[cs, :])
                    nc.scalar.dma_start(wv[:, ci * HS:(ci + 1) * HS], wv_d[cs, :])

                # warm the Exp table + the PE HAM clock-gate while DMAs run
                wsrc = pW.tile([P, NJ], F16, tag="wsrc")
                wdst = pW.tile([P, 8], F16, tag="wdst")
                wps = psA.tile([P, NJ], F32, tag="wps")
                nc.vector.memset(wsrc[:], 0.0)
                nc.scalar.activation(wdst[:], wsrc[:, 0:8], AF.Exp, scale=1.0)

                def warm_mm(n):
                    for _ in range(n):
                        nc.tensor.matmul(
                            wps[:], wsrc[:, 0:P], wsrc[:], start=True, stop=True
                        )

                warm_mm(12)
                for j0 in range(0, T, NJ):
                    ktp = psA.tile([HS, NJ], F32, tag="ktp")
                    for ci in range(NCC):
                        nc.tensor.matmul(
                            ktp[:],
                            wk[:, ci * HS:(ci + 1) * HS],
                            xt[:, ci * T + j0: ci * T + j0 + NJ],
                            start=(ci == 0),
                            stop=(ci == NCC - 1),
                        )
                    nc.vector.tensor_copy(kt[:, j0:j0 + NJ], ktp[:])
                    if j0 < T - 2 * NJ:
                        warm_mm(2)

                for j0 in range(0, TH, NJ):
                    qtp = psA.tile([HS, NJ], F32, tag="ktp")
                    for ci in range(NCC):
                        nc.tensor.matmul(
                            qtp[:],
                            wq[:, ci * HS:(ci + 1) * HS],
                            xt[:, ci * T + j0: ci * T + j0 + NJ],
                            start=(ci == 0),
                            stop=(ci == NCC - 1),
                        )
                    nc.vector.tensor_copy(qt[:, j0:j0 + NJ], qtp[:])

                vTsb = pA.tile([HS, TH], F16, tag="vTsb")
                for j0 in range(0, TH, NJ):
                    vtp = psA.tile([HS, NJ], F32, tag="ktp")
                    for ci in range(NCC):
                        nc.tensor.matmul(
                            vtp[:],
                            wv[:, ci * HS:(ci + 1) * HS],
                            xt[:, ci * T + j0: ci * T + j0 + NJ],
                            start=(ci == 0),
                            stop=(ci == NCC - 1),
                        )
                    nc.vector.tensor_copy(vTsb[:, j0:j0 + NJ], vtp[:])
                for k in range(NB):
                    vp = psA.tile([P, HS], F16, tag="vp")
                    nc.tensor.transpose(
                        vp[:], vTsb[:, k * P:(k + 1) * P], id16[:]
                    )
                    nc.vector.tensor_copy(v[:, k * HS:(k + 1) * HS], vp[:])

            # ---- phase B: scores + mask-bias inject, exp w/ accum ----
            with (
                tc.tile_pool(name="pmb", bufs=12) as pmb,
                tc.tile_pool(name="pBl", bufs=2) as pBl,
                tc.tile_pool(name="psB", bufs=2, space="PSUM") as psB,
            ):
                for k in range(NB):
                    mb = pmb.tile([P, T], F8, tag="mb")
                    nc.gpsimd.dma_start(mb[:, 0:TH], mb_d[k * P:(k + 1) * P, 0:TH])
                    nc.sync.dma_start(mb[:, TH:T], mb_d[k * P:(k + 1) * P, TH:T])
                    l_halves = []
                    for jh in range(2):
                        sp = psB.tile([P, TH], F32, tag="s")
                        for c in range(4):
                            nc.tensor.matmul(
                                sp[:, c * NJ:(c + 1) * NJ],
                                qt[:, k * P:(k + 1) * P],
                                kt[:, jh * TH + c * NJ: jh * TH + (c + 1) * NJ],
                                start=True,
                                stop=False,
                            )
                        for c in range(4):
                            nc.tensor.matmul(
                                sp[:, c * NJ:(c + 1) * NJ],
                                idt[:],
                                mb[:, jh * TH + c * NJ: jh * TH + (c + 1) * NJ],
                                start=False,
                                stop=True,
                            )
                        l_acc = pBl.tile([P, 1], F32, tag=f"l{jh}")
                        nc.scalar.activation(
                            u[:, k * T + jh * TH: k * T + (jh + 1) * TH],
                            sp[:],
                            AF.Exp,
                            scale=0.125,
                            accum_out=l_acc[:],
                        )
                        l_halves.append(l_acc)
                    l_tot = pBl.tile([P, 1], F32, tag="lt")
                    nc.vector.tensor_tensor(
                        out=l_tot[:], in0=l_halves[0][:], in1=l_halves[1][:],
                        op=ALU.add,
                    )
                    rl = pBl.tile([P, 1], F32, tag="rl")
                    nc.vector.reciprocal(rl[:], l_tot[:])
                    nc.vector.tensor_scalar(
                        out=vt[:, k * HS:(k + 1) * HS],
                        in0=v[:, k * HS:(k + 1) * HS],
                        scalar1=rl[:],
                        scalar2=VSCALE,
                        op0=ALU.mult,
                        op1=ALU.mult,
                    )

            # ---- phase C: OT = sum_k vt_k^T . u_k  (transposed AV) ----
            # j-chunk outer so each chunk's copy + store pipelines behind
            # the remaining chunks' matmuls.
            with (
                tc.tile_pool(name="psC", bufs=1, space="PSUM") as psC,
                tc.tile_pool(name="phC", bufs=1) as pC,
            ):
                ot = psC.tile([HS, T], F32, tag="ot")
                ot_sb = pC.tile([HS, T], F32, tag="ot_sb")
                for jq in range(T // NJ):
                    js = slice(jq * NJ, (jq + 1) * NJ)
                    for k in range(NB):
                        nc.tensor.matmul(
                            ot[:, js],
                            vt[:, k * HS:(k + 1) * HS],
                            u[:, k * T + jq * NJ: k * T + (jq + 1) * NJ],
                            start=(k == 0),
                            stop=(k == NB - 1),
                        )
                    if jq % 2 == 0:
                        nc.scalar.copy(ot_sb[:, js], ot[:, js])
                    else:
                        nc.vector.tensor_copy(ot_sb[:, js], ot[:, js])
                    if jq % 2 == 0:
                        nc.sync.dma_start(ot_d[:, js], ot_sb[:, js])
                    else:
                        nc.scalar.dma_start(ot_d[:, js], ot_sb[:, js])

    nc.compile()
    _CACHE["nc"] = nc
    return nc


def _prep_inputs(X, dag, Wk, Wq, Wv):
    X = np.asarray(X, dtype=np.float32)
    dag = np.asarray(dag)
    w16 = {
        "wk": np.asarray(Wk, dtype=np.float16),
        "wq": np.asarray(Wq, dtype=np.float16),
        "wv": np.asarray(Wv, dtype=np.float16),
    }
    idt = np.eye(P, dtype=ml_dtypes.float8_e4m3)
    id16 = np.eye(HS, dtype=np.float16)
    mbias = np.where(dag == 0, np.float32(MBIAS), np.float32(0.0)).astype(
        ml_dtypes.float8_e4m3
    )
    # column-permuted views: the core's own query-half tokens come first,
    # so qt/v matmuls can read fixed xt columns in the shared SPMD program.
    mb_perm = [
        np.ascontiguousarray(
            np.concatenate(
                [
                    mbias[h * TH:(h + 1) * TH, h * TH:(h + 1) * TH],
                    mbias[h * TH:(h + 1) * TH, (1 - h) * TH:(2 - h) * TH],
                ],
                axis=1,
            )
        )
        for h in range(2)
    ]
    in_maps = []
    for core in range(8):
        b, h = divmod(core, 2)
        xb = X[b].astype(np.float16)
        xb2 = np.concatenate(
            [xb[h * TH:(h + 1) * TH], xb[(1 - h) * TH:(2 - h) * TH]], axis=0
        )
        in_maps.append(
            {
                "xt": np.ascontiguousarray(xb2.T),
                "mb": mb_perm[h],
                "idt": idt,
                "id16": id16,
                **w16,
            }
        )
    return in_maps


def kernel(X, dag, Wk, Wq, Wv, _trace=False):
    nc = _build()
    in_maps = _prep_inputs(X, dag, Wk, Wq, Wv)
    res = run_bass_kernel_spmd(nc, in_maps, list(range(8)), trace=_trace)
    out = np.empty((B, T, HS), dtype=np.float32)
    for b in range(B):
        ot0 = res.results[2 * b]["ot"]          # h=0: columns already global
        ot1 = res.results[2 * b + 1]["ot"]      # h=1: halves swapped
        ot = ot0.copy()
        ot[:, 0:TH] += ot1[:, TH:T]
        ot[:, TH:T] += ot1[:, 0:TH]
        o = ot.T / np.float32(VSCALE)
        out[b] = o / (1.0 + np.exp(-o))  # swish: o * sigmoid(o)
    if _trace:
        return out, res
    return out
